# revision 48
# baseline (speedup 1.0000x reference)
"""Trainium2 Bass kernel for nn_Diagnet (S=1024, B=64, I=512, H=2048, O=512).

    u = einsum('sbi,hi->sbh', X, W_ih)
    h_t = |u_t + hh * h_{t-1}|   (scan over S, only final h needed)
    Y = h_final @ W_ho.T + b_ho

Strategy (8 NeuronCores, data-parallel over batch, 8 batch rows/core):

* H lanes permuted so hh is sorted descending, split into 16 chunks of
  128.  Chunk g only needs the last kg steps where amax(g)^kg ~ 1e-5
  (truncation, exact far below fp16 noise).  kg rounds up to 64-step
  blocks; chunks with kg == 64 are "shorts" (window = final block only).
* GEMM in fp16 (X, W_ih host-cast).  PSUM fp32, then the Activation
  engine copies each [128, (b,tau)] tile into a per-column fp16 u
  buffer.  X arrives in 4 large row-contiguous DMAs (block 15's tile
  first so the end-of-sequence work unblocks early).
* The scan runs on the DVE as a custom instruction ABS_SCAN_ANT:
      state_k = |state_{k-1} - u_k * scn_k|   (ABSOLUTE_DIFF prefix scan)
  with scn = NEGATED pre-scales -a^(K-1-t), so state_k tracks the
  pre-scaled recurrence m_t = a^(K-1-t) h_t and the final element IS
  h_final.  A mask (Idx >= K-1) + ADD-accum extracts the final state
  into m[:, (g,b)], which is also the s0 carry for the next piece of
  the same column.  One instruction covers up to a whole window.
* Shorts all merge into ONE scan stream per batch column: segments of
  [24 flush elements (POSITIVE scales 128*2^-j fold any state to
  <1e-5) + 64-step window].  Uniform 88-element segments put every
  chunk's final state at stride 88; one strided DVE copy gathers them
  into m.
* Block production order: 15 FIRST (it ends every window -> shorts and
  mid chunks unblock early), then the mid chunks' remaining blocks,
  then 0..11 ascending for chunk 0's piece-chasing.  The tail after
  the last GEMM is chunk 0's last piece + the final projection.
* Final projection: per chunk, m -> fp16 on the Activation engine,
  matmul vs fp16 W_ho^T accumulated in one PSUM bank, all issued at
  the very end (the PE runs in order - issuing them early would stall
  the PE queue on the DVE scan pipeline); bias added on DVE.
"""

import math
import os

from contextlib import ExitStack

import numpy as np

S, B, I, H, O = 1024, 64, 512, 2048, 512
NCORES = 8
BC = B // NCORES  # 8 batch rows per core
TB = 64  # time block
NBLK = S // TB  # 16
NCH = H // 128  # 16 h-chunks
NI = I // 128  # 4 i-chunks
XW = NI * TB * BC  # 2048 X cols per block (ic-major)
LN_TRUNC = 11.5  # a^K <= e^-11.5 ~ 1e-5 -> truncate (gate is 2e-2)
NFLUSH = 16  # 32*2^-16 ~ 5e-4 residual after flush (<< fp16 noise)
FLUSH_M = 32.0  # fold start; shorts' h stays well under 32
WB = 16  # tier-B short window (chunks whose raw k <= WB)

_CACHE = {}


def _register_abs_scan():
    import concourse.dve_ops as dve_ops
    from concourse.dve_spec import Spec, Src0, Src1, Zero, C0, C1, scan, Idx, lower, AluOp
    from concourse.dve_uop import DveOpSpec

    for op in dve_ops.OPS:
        if op.name == "ABS_SCAN_ANT":
            return op

    def ref(in0, in1, s0, s1, imm2):
        x = in0.astype(np.float32) * in1.astype(np.float32)
        st = np.broadcast_to(np.asarray(s0, np.float32), x[:, 0].shape).copy()
        out = np.empty_like(x, dtype=np.float32)
        for k in range(x.shape[-1]):
            st = np.abs(st - x[:, k])
            out[:, k] = st * (k >= s1)
        return out

    state = scan(AluOp.ABSOLUTE_DIFF, Src0 * Src1, init=C0)
    spec = Spec(body=state * (Idx >= C1), accum=AluOp.ADD, accum_init=Zero, reference=ref)
    row = max(dve_ops._SUB_OPCODE_FOR_NAME.values()) + 1
    assert row < 0x20
    shas = {}
    for ver in ("v3", "v4"):
        s = DveOpSpec(name="ABS_SCAN_ANT", opcode=row, uops=lower(spec, ver=ver), rd1_en=True)
        shas[ver] = s.sha(ver)
    op = dve_ops.DveOp("ABS_SCAN_ANT", spec, subdim=False, uops_sha=shas)
    dve_ops._SUB_OPCODE_FOR_NAME["ABS_SCAN_ANT"] = row
    dve_ops.OPS.append(op)
    dve_ops.CUSTOM_DVE_SPECS["ABS_SCAN_ANT"] = spec
    return op


def _make_plan(hh):
    a = np.maximum(np.abs(hh.astype(np.float64)), 1e-30)
    perm = np.argsort(-a, kind="stable")
    a_s = a[perm]
    kgs = []
    for g in range(NCH):
        amax = a_s[g * 128]
        if amax >= math.exp(-LN_TRUNC / S):
            kg = S
        else:
            kg = min(S, int(math.ceil(LN_TRUNC / math.log(1.0 / amax))))
        kg = max(TB, min(S, ((kg + TB - 1) // TB) * TB))
        kgs.append(kg)
    assert all(kgs[g] >= kgs[g + 1] for g in range(NCH - 1)), kgs
    ag = a_s.reshape(NCH, 128)  # [chunk, lane]

    longs = [g for g in range(NCH) if kgs[g] > TB]
    shorts = [g for g in range(NCH) if kgs[g] == TB]
    NSH = len(shorts)

    # Two-tier shorts: tier A keeps the full 64-step window; tier B
    # (chunks whose raw truncation k <= WB) uses a WB-step window read
    # from the tail of block 15.  Each segment starts with NFLUSH flush
    # elements (POSITIVE scales FLUSH_M*2^-j fold any carry to <1e-3).
    kraw = {}
    for g in shorts:
        amax = a_s[g * 128]
        kraw[g] = min(S, int(math.ceil(LN_TRUNC / math.log(1.0 / max(amax, 1e-12)))))
    tierA = [g for g in shorts if kraw[g] > WB]
    tierB = [g for g in shorts if kraw[g] <= WB]
    NA, NB = len(tierA), len(tierB)
    SEGA = NFLUSH + TB
    SEGB = NFLUSH + WB
    sh_cols = NA * SEGA + NB * SEGB
    scn_off = {}
    off = 0
    for g in longs:
        scn_off[g] = off
        off += kgs[g]
    scn_shorts_off = off
    scn_cols = off + sh_cols
    sh = np.zeros((128, max(sh_cols, 1)), dtype=np.float64)
    flush = FLUSH_M * (0.5 ** np.arange(NFLUSH))
    for i, g in enumerate(tierA):
        base = i * SEGA
        sh[:, base : base + NFLUSH] = flush[None, :]
        t = np.arange(TB)
        sh[:, base + NFLUSH : base + SEGA] = -(ag[g][:, None] ** (TB - 1 - t)[None, :])
    for i, g in enumerate(tierB):
        base = NA * SEGA + i * SEGB
        sh[:, base : base + NFLUSH] = flush[None, :]
        t = np.arange(WB)
        sh[:, base + NFLUSH : base + SEGB] = -(ag[g][:, None] ** (WB - 1 - t)[None, :])
    scn = sh.astype(np.float32)
    # per-long-chunk generator constants: 1/a and -a^kg
    aux = np.zeros((128, 2 * len(longs)), dtype=np.float64)
    for j, g in enumerate(longs):
        aux[:, 2 * j] = 1.0 / ag[g]
        aux[:, 2 * j + 1] = -(ag[g] ** kgs[g])
    aux = aux.astype(np.float32)


    fb = {g: NBLK - kgs[g] // TB for g in longs}
    fb0 = fb[longs[0]]
    # block production order: 15 first (ends every window), then chunk 0's
    # early blocks INTERLEAVED with the mid chunks' blocks so chunk 0's
    # scan chain starts chasing production immediately (its scans are the
    # tail otherwise), then the rest of chunk 0 ascending.
    mids = longs[1:]
    mid_lo = min((fb[g] for g in mids), default=NBLK - 1)
    order = [NBLK - 1]
    order += [kb for kb in range(mid_lo, NBLK - 1)]
    order += [kb for kb in range(fb0, mid_lo)]
    assert sorted(order) == list(range(fb0, NBLK)), (order, fb)

    # scan pieces: mids = one piece (their blocks all produced early);
    # chunk 0 split so pieces chase production, last piece covers the
    # late-produced blocks in one go.
    pieces = {}
    for g in mids:
        pieces[g] = [list(range(fb[g], NBLK))]
    # chunk 0: ONE scan per column.  The DVE is near-saturated from
    # ~30us on, so total scan work (262ns per-instruction overhead)
    # matters more than early firing; measured best.
    pieces[longs[0]] = [list(range(fb0, NBLK))]

    return {
        "perm": perm,
        "kgs": tuple(kgs),
        "longs": tuple(longs),
        "shorts": tuple(shorts),
        "scn_off": scn_off,
        "scn_shorts_off": scn_shorts_off,
        "tierA": tuple(tierA),
        "tierB": tuple(tierB),
        "sh_cols": sh_cols,
        "fb": fb,
        "order": tuple(order),
        "pieces": pieces,
        "SCN": scn,
        "AUX": aux,
        "scn_cols": scn_cols,
    }


def _build(plan):
    import concourse.mybir as mybir
    import concourse.tile as tile
    from concourse import bacc
    from concourse.bass import ds

    ABS_SCAN = _register_abs_scan()
    f32 = mybir.dt.float32
    f16 = mybir.dt.float16

    kgs = plan["kgs"]
    longs = plan["longs"]
    shorts = plan["shorts"]
    tierA = plan["tierA"]
    tierB = plan["tierB"]
    NA, NB = len(tierA), len(tierB)
    SEGA = NFLUSH + TB
    SEGB = NFLUSH + WB
    sh_cols = plan["sh_cols"]
    NSH = len(shorts)
    scn_off = plan["scn_off"]
    scn_shorts_off = plan["scn_shorts_off"]
    fb = plan["fb"]
    order = plan["order"]
    pieces = plan["pieces"]
    scn_cols = plan["scn_cols"]
    NLG = len(longs)
    NORD = len(order)
    pos = {kb: i for i, kb in enumerate(order)}
    # map block -> (chunk g, piece index, offset-in-piece)
    piece_of = {}
    for g in longs:
        for pi, blks in enumerate(pieces[g]):
            for kb in blks:
                piece_of[(g, kb)] = (pi, blks.index(kb), len(blks))

    nc = bacc.Bacc("TRN2", target_bir_lowering=False, debug=False, num_devices=NCORES)
    # X rows are production-ordered: X[p, i, :] = block order[i], ic-major.
    X = nc.dram_tensor("X", [128, NORD, XW], f16, kind="ExternalInput").ap()
    WIHT = nc.dram_tensor("WIHT", [128, NI * H], f16, kind="ExternalInput").ap()
    WHOT = nc.dram_tensor("WHOT", [128, NCH * O], f16, kind="ExternalInput").ap()
    BIAS = nc.dram_tensor("BIAS", [128, O + BC], f16, kind="ExternalInput").ap()
    SCN = nc.dram_tensor("SCN", [128, plan["SCN"].shape[1]], f32, kind="ExternalInput").ap()
    AUX = nc.dram_tensor("AUX", [128, 2 * NLG], f32, kind="ExternalInput").ap()
    Y = nc.dram_tensor("Y", [BC, O], f32, kind="ExternalOutput").ap()

    with tile.TileContext(nc) as tc:
        with ExitStack() as ctx:
            consts = ctx.enter_context(tc.tile_pool(name="consts", bufs=1))
            gpool = ctx.enter_context(tc.tile_pool(name="gpsum", bufs=7, space="PSUM"))
            fpool = ctx.enter_context(tc.tile_pool(name="fpsum", bufs=1, space="PSUM"))

            # Dependency-granular tiles: one tile per DMA / per consumer so
            # Tile's coarse hazard tracking never falsely serializes.
            # Alternate the two HW DGE queues (SP is "sync", Act "scalar").
            qs = [nc.scalar, nc.sync]

            # Act queue gets ONLY a few early never-blocking DMAs (its
            # sequencer also runs the PSUM->SBUF copies; DMA-issue
            # instructions block on completion-semaphore reuse).  Sync
            # carries the bulk; its stalls block nothing.
            wih = [consts.tile([128, H], f16, tag=f"wih{ic}", name=f"wih{ic}") for ic in range(NI)]
            nc.scalar.dma_start(wih[0][:], WIHT[:, ds(0, H)])
            xt = {}
            for i, kb in enumerate(order):
                t = consts.tile([128, XW], f16, tag=f"xt{kb}", name=f"xt{kb}")
                xt[kb] = t
                (nc.scalar if i == 0 else nc.sync).dma_start(t[:], X[:, ds(i, 1), :])
                if i == 0:
                    if NSH:
                        scn_sh = consts.tile(
                            [128, sh_cols], f32, tag="scnsh", name="scn_sh"
                        )
                        nc.scalar.dma_start(scn_sh[:], SCN)
                    for ic in range(1, NI):
                        nc.sync.dma_start(wih[ic][:], WIHT[:, ds(ic * H, H)])
                    aux_t = consts.tile([128, 2 * NLG], f32, tag="aux", name="aux_t")
                    nc.sync.dma_start(aux_t[:], AUX)
            bias_t = consts.tile([128, O + BC], f16, tag="bias", name="bias_t")
            nc.sync.dma_start(bias_t[:], BIAS)
            who_t = consts.tile([128, NCH * O], f16, tag="whot", name="who_t")
            nc.sync.dma_start(who_t[:], WHOT)

            # u tiles: one per (long chunk, piece) + one for the shorts
            ut = {}
            for g in longs:
                for pi, blks in enumerate(pieces[g]):
                    plen = len(blks) * TB
                    ut[(g, pi)] = consts.tile(
                        [128, BC * plen], f16, tag=f"u{g}_{pi}", name=f"u{g}_{pi}"
                    )
            if NSH:
                ush = consts.tile([128, BC * sh_cols], f16, tag="ush", name="ush")
            m_t = consts.tile([128, NCH * BC], f32, tag="m", name="m_t")
            mh_t = consts.tile([128, NCH * BC], f16, tag="mh", name="mh_t")
            scr = consts.tile([128, max(sh_cols, S)], f32, tag="scr", name="scr")
            nc.vector.memset(m_t[:], 0.0)
            if NSH:
                # whole shorts u region -> 1.0: flush cells keep it (their
                # scales carry the flush constants); the copies overwrite
                # the window cells afterwards
                nc.gpsimd.memset(ush[:], 1.0)

            # long-chunk scales generated on-chip (geometric series)
            scn_lg = consts.tile([128, scn_cols - sh_cols], f32, tag="scnlg", name="scn_lg") if NLG else None
            zc = consts.tile([128, 1], f32, tag="zc", name="zc")
            nc.vector.memset(zc[:], 0.0)
            for j, g in enumerate(longs):
                kg = kgs[g]
                nc.vector.tensor_tensor_scan(
                    scn_lg[:, ds(scn_off[g], kg)],
                    aux_t[:, ds(2 * j, 1)].broadcast_to([128, kg]),
                    zc[:].broadcast_to([128, kg]),
                    aux_t[:, ds(2 * j + 1, 1)],
                    mybir.AluOpType.mult,
                    mybir.AluOpType.add,
                )

            psy = fpool.tile([BC, O], f32, tag="fy", name="psy")

            def produce(kb):
                active = [g for g in longs if fb[g] <= kb]
                if kb == NBLK - 1:
                    # shorts first: their copies unblock the merged scan
                    active = list(shorts) + active
                for lo in range(0, len(active), 7):
                    grp = active[lo : lo + 7]
                    ps = {
                        g: gpool.tile([128, TB * BC], f32, tag="gp", name=f"gp_{kb}_{g}")
                        for g in grp
                    }
                    for ic in range(NI):
                        for g in grp:
                            nc.tensor.matmul(
                                ps[g][:],
                                wih[ic][:, ds(g * 128, 128)],
                                xt[kb][:, ds(ic * TB * BC, TB * BC)],
                                start=(ic == 0),
                                stop=(ic == NI - 1),
                            )
                    for g in grp:
                        src = ps[g][:].rearrange("p (b t) -> p b t", b=BC)
                        if g in tierA:
                            i = tierA.index(g)
                            dst = ush[:].rearrange(
                                "p (b t) -> p b t", b=BC, t=sh_cols
                            )[:, :, ds(i * SEGA + NFLUSH, TB)]
                        elif g in tierB:
                            i = tierB.index(g)
                            dst = ush[:].rearrange(
                                "p (b t) -> p b t", b=BC, t=sh_cols
                            )[:, :, ds(NA * SEGA + i * SEGB + NFLUSH, WB)]
                            src = src[:, :, ds(TB - WB, WB)]
                        else:
                            pi, idx, nblk = piece_of[(g, kb)]
                            plen = nblk * TB
                            dst = ut[(g, pi)][:].rearrange(
                                "p (b t) -> p b t", b=BC, t=plen
                            )[:, :, ds(idx * TB, TB)]
                        nc.scalar.copy(dst, src)

            def scan_piece(g, pi, blks):
                kg = kgs[g]
                lo = (blks[0] - fb[g]) * TB
                n = len(blks) * TB
                for b in range(BC):
                    nc.vector._custom_dve(
                        ABS_SCAN,
                        out=scr[:, ds(0, n)],
                        in0=ut[(g, pi)][:, ds(b * n, n)],
                        in1=scn_lg[:, ds(scn_off[g] + lo, n)],
                        s0=m_t[:, ds(g * BC + b, 1)],
                        s1=float(n - 1),
                        accum_out=m_t[:, ds(g * BC + b, 1)],
                    )

            def scan_tier(chunks, seg, col_lo, ncols):
                # one merged stream per batch column for a uniform-segment
                # tier; every segment starts with a flush, so the s0 carry
                # value is irrelevant and tiers are independent.
                cnt = len(chunks)
                g0 = chunks[0]
                for b in range(BC):
                    nc.vector._custom_dve(
                        ABS_SCAN,
                        out=scr[:, ds(0, ncols)],
                        in0=ush[:, ds(b * sh_cols + col_lo, ncols)],
                        in1=scn_sh[:, ds(col_lo, ncols)],
                        s0=m_t[:, ds(g0 * BC + b, 1)],
                        s1=float(-1.0),  # no mask; finals gathered from scr
                    )
                    src = scr[:, ds(0, ncols)].rearrange(
                        "p (s o) -> p s o", s=cnt, o=seg
                    )[:, :, ds(seg - 1, 1)]
                    dst = m_t[:, ds(g0 * BC, cnt * BC)].rearrange(
                        "p (s o) -> p s o", s=cnt, o=BC
                    )[:, :, ds(b, 1)]
                    nc.vector.tensor_scalar_mul(dst, src, 1.0)

            # ---- schedule ----
            produced = []
            scanned_pieces = {g: 0 for g in longs}
            shorts_done = [False]

            def try_scans(at_end):
                if not shorts_done[0] and NBLK - 1 in produced and NSH:
                    if NA:
                        scan_tier(tierA, SEGA, 0, NA * SEGA)
                    if NB:
                        scan_tier(tierB, SEGB, NA * SEGA, NB * SEGB)
                    shorts_done[0] = True
                for g in longs:
                    grps = pieces[g]
                    while scanned_pieces[g] < len(grps):
                        pi = scanned_pieces[g]
                        blks = grps[pi]
                        if not all(kb in produced for kb in blks):
                            break
                        scan_piece(g, pi, blks)
                        scanned_pieces[g] += 1

            for kb in order:
                produce(kb)
                produced.append(kb)
                try_scans(False)
            try_scans(True)
            assert shorts_done[0] or not NSH
            assert all(scanned_pieces[g] == len(pieces[g]) for g in longs)

            # final projection, all deferred here (PE is in-order); the
            # chunk with the tail scan goes last.
            fin_order = list(longs[1:]) + list(shorts) + [longs[0]]
            # bias enters as a rank-1 matmul: stationary = 1/128 columns,
            # moving = bias broadcast over partitions -> psy[b,o] = b_ho[o]
            nc.tensor.matmul(
                psy[:],
                bias_t[:, ds(O, BC)],
                bias_t[:, ds(0, O)],
                start=True,
                stop=False,
                skip_group_check=True,
            )
            for i, g in enumerate(fin_order):
                nc.scalar.copy(mh_t[:, ds(g * BC, BC)], m_t[:, ds(g * BC, BC)])
                nc.tensor.matmul(
                    psy[:],
                    mh_t[:, ds(g * BC, BC)],
                    who_t[:, ds(g * O, O)],
                    start=False,
                    stop=(i == NCH - 1),
                    skip_group_check=True,
                )

            y_t = consts.tile([BC, O], f32, tag="y", name="y_t")
            nc.scalar.copy(y_t[:], psy[:])
            nc.scalar.dma_start(Y, y_t[:])  # fast queue; sync's crawls
    nc.compile()
    return nc


def _get_program(plan):
    key = (plan["kgs"], plan["longs"])
    if key not in _CACHE:
        _CACHE[key] = _build(plan)
    return _CACHE[key]


def _ensure_ntff_hook():
    """Provide antenv.axon_hooks (absent in this image) so trace=True works."""
    import sys
    import types

    if "antenv.axon_hooks" in sys.modules:
        return True
    try:
        import antenv

        mod = types.ModuleType("antenv.axon_hooks")
        mod._hook = None

        def set_axon_ntff_profile_hook(h):
            mod._hook = h

        def get_axon_ntff_profile_hook():
            return mod._hook

        mod.set_axon_ntff_profile_hook = set_axon_ntff_profile_hook
        mod.get_axon_ntff_profile_hook = get_axon_ntff_profile_hook
        sys.modules["antenv.axon_hooks"] = mod
        antenv.axon_hooks = mod

        from trn_agent_boot.trn_boot import _ntff_profile_via_ctypes

        hook = _ntff_profile_via_ctypes("/opt/axon/libaxon_pjrt.so")
        mod.set_axon_ntff_profile_hook(hook)
        return hook is not None
    except Exception:
        return False


def kernel(X, W_ih, hh, W_ho, b_ho):
    from concourse import bass_utils

    X = np.asarray(X, dtype=np.float32)
    W_ih = np.asarray(W_ih, dtype=np.float32)
    hh = np.asarray(hh, dtype=np.float32)
    W_ho = np.asarray(W_ho, dtype=np.float32)
    b_ho = np.asarray(b_ho, dtype=np.float32)

    plan = _make_plan(hh)
    perm = plan["perm"]
    order = plan["order"]
    nc = _get_program(plan)

    wiht = np.ascontiguousarray(W_ih[perm].T).astype(np.float16)  # [I, H]
    wiht = np.ascontiguousarray(
        wiht.reshape(NI, 128, H).transpose(1, 0, 2).reshape(128, NI * H)
    )
    whot = np.ascontiguousarray(W_ho[:, perm].T).astype(np.float16)  # [H, O]
    whot = np.ascontiguousarray(
        whot.reshape(NCH, 128, O).transpose(1, 0, 2).reshape(128, NCH * O)
    )
    bias = np.zeros((128, O + BC), dtype=np.float16)
    bias[:, :O] = b_ho[None, :].astype(np.float16)
    bias[:, O:] = np.float16(1.0 / 128.0)

    common = {
        "WIHT": wiht,
        "WHOT": whot,
        "BIAS": bias,
        "SCN": plan["SCN"],
        "AUX": plan["AUX"],
    }
    in_maps = []
    for m in range(NCORES):
        im = dict(common)
        xm = X[:, m * BC : (m + 1) * BC, :]  # [S, BC, I]
        # per block: [NI, 128, BC*TB] -> row-major [128, NI*TB*BC]
        xt = xm.transpose(2, 1, 0).reshape(NI, 128, BC, NBLK, TB)
        xt = xt.transpose(3, 1, 0, 2, 4).reshape(NBLK, 128, XW)
        xt = xt[list(order)]  # production order
        im["X"] = np.ascontiguousarray(xt.transpose(1, 0, 2)).astype(np.float16)
        in_maps.append(im)

    trace = bool(int(os.environ.get("DIAG_TRACE", "0")))
    if trace:
        trace = _ensure_ntff_hook()
    res = None
    for attempt in range(3):
        try:
            res = bass_utils.run_bass_kernel_spmd(
                nc,
                in_maps,
                core_ids=list(range(NCORES)),
                trace=trace,
                tmpdir=os.environ.get("DIAG_TRACE_DIR") or None,
            )
            break
        except Exception:
            if attempt == 2:
                raise
            trace = False  # retry without profiling
    if res.exec_time_ns is not None:
        kernel.last_exec_time_ns = res.exec_time_ns
        kernel.last_mean_exec_time_ns = res.mean_exec_time_ns
    Yfull = np.concatenate([r["Y"] for r in res.results], axis=0)
    return Yfull


kernel.last_exec_time_ns = None
kernel.last_mean_exec_time_ns = None


# revision 50
# speedup vs baseline: 1.0266x; 1.0266x over previous
"""Trainium2 Bass kernel for nn_Diagnet (S=1024, B=64, I=512, H=2048, O=512).

    u = einsum('sbi,hi->sbh', X, W_ih)
    h_t = |u_t + hh * h_{t-1}|   (scan over S, only final h needed)
    Y = h_final @ W_ho.T + b_ho

Strategy (8 NeuronCores, data-parallel over batch, 8 batch rows/core):

* H lanes permuted so hh is sorted descending, split into 16 chunks of
  128.  Chunk g only needs the last kg steps where amax(g)^kg ~ 1e-5
  (truncation, exact far below fp16 noise).  kg rounds up to 64-step
  blocks; chunks with kg == 64 are "shorts" (window = final block only).
* GEMM in fp16 (X, W_ih host-cast).  PSUM fp32, then the Activation
  engine copies each [128, (b,tau)] tile into a per-column fp16 u
  buffer.  X arrives in 4 large row-contiguous DMAs (block 15's tile
  first so the end-of-sequence work unblocks early).
* The scan runs on the DVE as a custom instruction ABS_SCAN_ANT:
      state_k = |state_{k-1} - u_k * scn_k|   (ABSOLUTE_DIFF prefix scan)
  with scn = NEGATED pre-scales -a^(K-1-t), so state_k tracks the
  pre-scaled recurrence m_t = a^(K-1-t) h_t and the final element IS
  h_final.  A mask (Idx >= K-1) + ADD-accum extracts the final state
  into m[:, (g,b)], which is also the s0 carry for the next piece of
  the same column.  One instruction covers up to a whole window.
* Shorts all merge into ONE scan stream per batch column: segments of
  [24 flush elements (POSITIVE scales 128*2^-j fold any state to
  <1e-5) + 64-step window].  Uniform 88-element segments put every
  chunk's final state at stride 88; one strided DVE copy gathers them
  into m.
* Block production order: 15 FIRST (it ends every window -> shorts and
  mid chunks unblock early), then the mid chunks' remaining blocks,
  then 0..11 ascending for chunk 0's piece-chasing.  The tail after
  the last GEMM is chunk 0's last piece + the final projection.
* Final projection: per chunk, m -> fp16 on the Activation engine,
  matmul vs fp16 W_ho^T accumulated in one PSUM bank, all issued at
  the very end (the PE runs in order - issuing them early would stall
  the PE queue on the DVE scan pipeline); bias added on DVE.
"""

import math
import os

from contextlib import ExitStack

import numpy as np

S, B, I, H, O = 1024, 64, 512, 2048, 512
NCORES = 8
BC = B // NCORES  # 8 batch rows per core
TB = 64  # time block
NBLK = S // TB  # 16
NCH = H // 128  # 16 h-chunks
NI = I // 128  # 4 i-chunks
XW = NI * TB * BC  # 2048 X cols per block (ic-major)
LN_TRUNC = 11.5  # a^K <= e^-11.5 ~ 1e-5 -> truncate (gate is 2e-2)
NFLUSH = 16  # 32*2^-16 ~ 5e-4 residual after flush (<< fp16 noise)
FLUSH_M = 32.0  # fold start; shorts' h stays well under 32
WB = 16  # tier-B short window (chunks whose raw k <= WB)

_CACHE = {}


def _register_abs_scan():
    import concourse.dve_ops as dve_ops
    from concourse.dve_spec import Spec, Src0, Src1, Zero, C0, C1, scan, Idx, lower, AluOp
    from concourse.dve_uop import DveOpSpec

    for op in dve_ops.OPS:
        if op.name == "ABS_SCAN_ANT":
            return op

    def ref(in0, in1, s0, s1, imm2):
        x = in0.astype(np.float32) * in1.astype(np.float32)
        st = np.broadcast_to(np.asarray(s0, np.float32), x[:, 0].shape).copy()
        out = np.empty_like(x, dtype=np.float32)
        for k in range(x.shape[-1]):
            st = np.abs(st - x[:, k])
            out[:, k] = st * (k >= s1)
        return out

    state = scan(AluOp.ABSOLUTE_DIFF, Src0 * Src1, init=C0)
    spec = Spec(body=state * (Idx >= C1), accum=AluOp.ADD, accum_init=Zero, reference=ref)
    row = max(dve_ops._SUB_OPCODE_FOR_NAME.values()) + 1
    assert row < 0x20
    shas = {}
    for ver in ("v3", "v4"):
        s = DveOpSpec(name="ABS_SCAN_ANT", opcode=row, uops=lower(spec, ver=ver), rd1_en=True)
        shas[ver] = s.sha(ver)
    op = dve_ops.DveOp("ABS_SCAN_ANT", spec, subdim=False, uops_sha=shas)
    dve_ops._SUB_OPCODE_FOR_NAME["ABS_SCAN_ANT"] = row
    dve_ops.OPS.append(op)
    dve_ops.CUSTOM_DVE_SPECS["ABS_SCAN_ANT"] = spec
    return op


def _make_plan(hh):
    a = np.maximum(np.abs(hh.astype(np.float64)), 1e-30)
    perm = np.argsort(-a, kind="stable")
    a_s = a[perm]
    kgs = []
    for g in range(NCH):
        amax = a_s[g * 128]
        if amax >= math.exp(-LN_TRUNC / S):
            kg = S
        else:
            kg = min(S, int(math.ceil(LN_TRUNC / math.log(1.0 / amax))))
        kg = max(TB, min(S, ((kg + TB - 1) // TB) * TB))
        kgs.append(kg)
    assert all(kgs[g] >= kgs[g + 1] for g in range(NCH - 1)), kgs
    ag = a_s.reshape(NCH, 128)  # [chunk, lane]

    longs = [g for g in range(NCH) if kgs[g] > TB]
    shorts = [g for g in range(NCH) if kgs[g] == TB]
    NSH = len(shorts)

    # Two-tier shorts: tier A keeps the full 64-step window; tier B
    # (chunks whose raw truncation k <= WB) uses a WB-step window read
    # from the tail of block 15.  Each segment starts with NFLUSH flush
    # elements (POSITIVE scales FLUSH_M*2^-j fold any carry to <1e-3).
    kraw = {}
    for g in shorts:
        amax = a_s[g * 128]
        kraw[g] = min(S, int(math.ceil(LN_TRUNC / math.log(1.0 / max(amax, 1e-12)))))
    tierA = [g for g in shorts if kraw[g] > WB]
    tierB = [g for g in shorts if kraw[g] <= WB]
    NA, NB = len(tierA), len(tierB)
    SEGA = NFLUSH + TB
    SEGB = NFLUSH + WB
    sh_cols = NA * SEGA + NB * SEGB
    scn_off = {}
    off = 0
    for g in longs:
        scn_off[g] = off
        off += kgs[g]
    scn_shorts_off = off
    scn_cols = off + sh_cols
    sh = np.zeros((128, max(sh_cols, 1)), dtype=np.float64)
    flush = FLUSH_M * (0.5 ** np.arange(NFLUSH))
    for i, g in enumerate(tierA):
        base = i * SEGA
        sh[:, base : base + NFLUSH] = flush[None, :]
        t = np.arange(TB)
        sh[:, base + NFLUSH : base + SEGA] = -(ag[g][:, None] ** (TB - 1 - t)[None, :])
    for i, g in enumerate(tierB):
        base = NA * SEGA + i * SEGB
        sh[:, base : base + NFLUSH] = flush[None, :]
        t = np.arange(WB)
        sh[:, base + NFLUSH : base + SEGB] = -(ag[g][:, None] ** (WB - 1 - t)[None, :])
    scn = sh.astype(np.float32)
    # per-long-chunk generator constants: 1/a and -a^kg
    aux = np.zeros((128, 2 * len(longs)), dtype=np.float64)
    for j, g in enumerate(longs):
        aux[:, 2 * j] = 1.0 / ag[g]
        aux[:, 2 * j + 1] = -(ag[g] ** kgs[g])
    aux = aux.astype(np.float32)


    fb = {g: NBLK - kgs[g] // TB for g in longs}
    fb0 = fb[longs[0]]
    # block production order: 15 first (ends every window), then chunk 0's
    # early blocks INTERLEAVED with the mid chunks' blocks so chunk 0's
    # scan chain starts chasing production immediately (its scans are the
    # tail otherwise), then the rest of chunk 0 ascending.
    mids = longs[1:]
    mid_lo = min((fb[g] for g in mids), default=NBLK - 1)
    order = [NBLK - 1]
    order += [kb for kb in range(mid_lo, NBLK - 1)]
    order += [kb for kb in range(fb0, mid_lo)]
    assert sorted(order) == list(range(fb0, NBLK)), (order, fb)

    # scan pieces: mids = one piece (their blocks all produced early);
    # chunk 0 split so pieces chase production, last piece covers the
    # late-produced blocks in one go.
    pieces = {}
    for g in mids:
        pieces[g] = [list(range(fb[g], NBLK))]
    # chunk 0: ONE scan per column.  The DVE is near-saturated from
    # ~30us on, so total scan work (262ns per-instruction overhead)
    # matters more than early firing; measured best.
    pieces[longs[0]] = [list(range(fb0, NBLK))]

    return {
        "perm": perm,
        "kgs": tuple(kgs),
        "longs": tuple(longs),
        "shorts": tuple(shorts),
        "scn_off": scn_off,
        "scn_shorts_off": scn_shorts_off,
        "tierA": tuple(tierA),
        "tierB": tuple(tierB),
        "sh_cols": sh_cols,
        "fb": fb,
        "order": tuple(order),
        "pieces": pieces,
        "SCN": scn,
        "AUX": aux,
        "scn_cols": scn_cols,
    }


def _build(plan):
    import concourse.mybir as mybir
    import concourse.tile as tile
    from concourse import bacc
    from concourse.bass import ds

    ABS_SCAN = _register_abs_scan()
    f32 = mybir.dt.float32
    f16 = mybir.dt.float16

    kgs = plan["kgs"]
    longs = plan["longs"]
    shorts = plan["shorts"]
    tierA = plan["tierA"]
    tierB = plan["tierB"]
    NA, NB = len(tierA), len(tierB)
    SEGA = NFLUSH + TB
    SEGB = NFLUSH + WB
    sh_cols = plan["sh_cols"]
    NSH = len(shorts)
    scn_off = plan["scn_off"]
    scn_shorts_off = plan["scn_shorts_off"]
    fb = plan["fb"]
    order = plan["order"]
    pieces = plan["pieces"]
    scn_cols = plan["scn_cols"]
    NLG = len(longs)
    NORD = len(order)
    pos = {kb: i for i, kb in enumerate(order)}
    # map block -> (chunk g, piece index, offset-in-piece)
    piece_of = {}
    for g in longs:
        for pi, blks in enumerate(pieces[g]):
            for kb in blks:
                piece_of[(g, kb)] = (pi, blks.index(kb), len(blks))

    nc = bacc.Bacc("TRN2", target_bir_lowering=False, debug=False, num_devices=NCORES)
    # X rows are production-ordered: X[p, i, :] = block order[i], ic-major.
    X = nc.dram_tensor("X", [128, NORD, XW], f16, kind="ExternalInput").ap()
    WIHT = nc.dram_tensor("WIHT", [128, NI * H], f16, kind="ExternalInput").ap()
    WHOT = nc.dram_tensor("WHOT", [128, NCH * O], f16, kind="ExternalInput").ap()
    BIAS = nc.dram_tensor("BIAS", [128, O + BC], f16, kind="ExternalInput").ap()
    SCN = nc.dram_tensor("SCN", [128, plan["SCN"].shape[1]], f32, kind="ExternalInput").ap()
    AUX = nc.dram_tensor("AUX", [128, 2 * NLG], f32, kind="ExternalInput").ap()
    Y = nc.dram_tensor("Y", [BC, O], f32, kind="ExternalOutput").ap()

    with tile.TileContext(nc) as tc:
        with ExitStack() as ctx:
            consts = ctx.enter_context(tc.tile_pool(name="consts", bufs=1))
            gpool = ctx.enter_context(tc.tile_pool(name="gpsum", bufs=7, space="PSUM"))
            fpool = ctx.enter_context(tc.tile_pool(name="fpsum", bufs=1, space="PSUM"))

            # Dependency-granular tiles: one tile per DMA / per consumer so
            # Tile's coarse hazard tracking never falsely serializes.
            # Alternate the two HW DGE queues (SP is "sync", Act "scalar").
            qs = [nc.scalar, nc.sync]

            # Act queue gets ONLY a few early never-blocking DMAs (its
            # sequencer also runs the PSUM->SBUF copies; DMA-issue
            # instructions block on completion-semaphore reuse).  Sync
            # carries the bulk; its stalls block nothing.
            wih = [consts.tile([128, H], f16, tag=f"wih{ic}", name=f"wih{ic}") for ic in range(NI)]
            nc.scalar.dma_start(wih[0][:], WIHT[:, ds(0, H)])
            xt = {}
            for i, kb in enumerate(order):
                t = consts.tile([128, XW], f16, tag=f"xt{kb}", name=f"xt{kb}")
                xt[kb] = t
                (nc.scalar if i == 0 else nc.sync).dma_start(t[:], X[:, ds(i, 1), :])
                if i == 0:
                    if NSH:
                        scn_sh = consts.tile(
                            [128, sh_cols], f32, tag="scnsh", name="scn_sh"
                        )
                        nc.scalar.dma_start(scn_sh[:], SCN)
                    for ic in range(1, NI):
                        nc.sync.dma_start(wih[ic][:], WIHT[:, ds(ic * H, H)])
                    aux_t = consts.tile([128, 2 * NLG], f32, tag="aux", name="aux_t")
                    nc.sync.dma_start(aux_t[:], AUX)
            bias_t = consts.tile([128, O + BC], f16, tag="bias", name="bias_t")
            nc.sync.dma_start(bias_t[:], BIAS)
            who_t = consts.tile([128, NCH * O], f16, tag="whot", name="who_t")
            nc.sync.dma_start(who_t[:], WHOT)

            # u tiles: one per (long chunk, piece) + one for the shorts
            ut = {}
            for g in longs:
                for pi, blks in enumerate(pieces[g]):
                    plen = len(blks) * TB
                    ut[(g, pi)] = consts.tile(
                        [128, BC * plen], f16, tag=f"u{g}_{pi}", name=f"u{g}_{pi}"
                    )
            if NSH:
                ush = consts.tile([128, BC * sh_cols], f16, tag="ush", name="ush")
            m_t = consts.tile([128, NCH * BC], f32, tag="m", name="m_t")
            mh_t = consts.tile([128, NCH * BC], f16, tag="mh", name="mh_t")
            scr = consts.tile([128, max(sh_cols, S)], f32, tag="scr", name="scr")
            nc.vector.memset(m_t[:], 0.0)
            if NSH:
                # whole shorts u region -> 1.0: flush cells keep it (their
                # scales carry the flush constants); the copies overwrite
                # the window cells afterwards
                nc.gpsimd.memset(ush[:], 1.0)

            # long-chunk scales generated on-chip (geometric series)
            scn_lg = consts.tile([128, scn_cols - sh_cols], f32, tag="scnlg", name="scn_lg") if NLG else None
            zc = consts.tile([128, 1], f32, tag="zc", name="zc")
            nc.vector.memset(zc[:], 0.0)
            for j, g in enumerate(longs):
                kg = kgs[g]
                nc.vector.tensor_tensor_scan(
                    scn_lg[:, ds(scn_off[g], kg)],
                    aux_t[:, ds(2 * j, 1)].broadcast_to([128, kg]),
                    zc[:].broadcast_to([128, kg]),
                    aux_t[:, ds(2 * j + 1, 1)],
                    mybir.AluOpType.mult,
                    mybir.AluOpType.add,
                )

            psy = fpool.tile([BC, O], f32, tag="fy", name="psy")

            def produce(kb):
                active = [g for g in longs if fb[g] <= kb]
                if kb == NBLK - 1:
                    # shorts first: their copies unblock the merged scan
                    active = list(shorts) + active
                for lo in range(0, len(active), 7):
                    grp = active[lo : lo + 7]
                    ps = {
                        g: gpool.tile([128, TB * BC], f32, tag="gp", name=f"gp_{kb}_{g}")
                        for g in grp
                    }
                    for ic in range(NI):
                        for g in grp:
                            nc.tensor.matmul(
                                ps[g][:],
                                wih[ic][:, ds(g * 128, 128)],
                                xt[kb][:, ds(ic * TB * BC, TB * BC)],
                                start=(ic == 0),
                                stop=(ic == NI - 1),
                            )
                    for g in grp:
                        src = ps[g][:].rearrange("p (b t) -> p b t", b=BC)
                        if g in tierA:
                            i = tierA.index(g)
                            dst = ush[:].rearrange(
                                "p (b t) -> p b t", b=BC, t=sh_cols
                            )[:, :, ds(i * SEGA + NFLUSH, TB)]
                        elif g in tierB:
                            i = tierB.index(g)
                            dst = ush[:].rearrange(
                                "p (b t) -> p b t", b=BC, t=sh_cols
                            )[:, :, ds(NA * SEGA + i * SEGB + NFLUSH, WB)]
                            src = src[:, :, ds(TB - WB, WB)]
                        else:
                            pi, idx, nblk = piece_of[(g, kb)]
                            plen = nblk * TB
                            dst = ut[(g, pi)][:].rearrange(
                                "p (b t) -> p b t", b=BC, t=plen
                            )[:, :, ds(idx * TB, TB)]
                        nc.scalar.copy(dst, src)

            def scan_piece(g, pi, blks):
                kg = kgs[g]
                lo = (blks[0] - fb[g]) * TB
                n = len(blks) * TB
                for b in range(BC):
                    nc.vector._custom_dve(
                        ABS_SCAN,
                        out=scr[:, ds(0, n)],
                        in0=ut[(g, pi)][:, ds(b * n, n)],
                        in1=scn_lg[:, ds(scn_off[g] + lo, n)],
                        s0=m_t[:, ds(g * BC + b, 1)],
                        s1=float(n - 1),
                        accum_out=m_t[:, ds(g * BC + b, 1)],
                    )

            def scan_tier(chunks, seg, col_lo, ncols):
                # one merged stream per batch column for a uniform-segment
                # tier; every segment starts with a flush, so the s0 carry
                # value is irrelevant and tiers are independent.
                cnt = len(chunks)
                g0 = chunks[0]
                for b in range(BC):
                    nc.vector._custom_dve(
                        ABS_SCAN,
                        out=scr[:, ds(0, ncols)],
                        in0=ush[:, ds(b * sh_cols + col_lo, ncols)],
                        in1=scn_sh[:, ds(col_lo, ncols)],
                        s0=m_t[:, ds(g0 * BC + b, 1)],
                        s1=float(-1.0),  # no mask; finals gathered from scr
                    )
                    src = scr[:, ds(0, ncols)].rearrange(
                        "p (s o) -> p s o", s=cnt, o=seg
                    )[:, :, ds(seg - 1, 1)]
                    dst = m_t[:, ds(g0 * BC, cnt * BC)].rearrange(
                        "p (s o) -> p s o", s=cnt, o=BC
                    )[:, :, ds(b, 1)]
                    nc.vector.tensor_scalar_mul(dst, src, 1.0)

            # ---- schedule ----
            produced = []
            scanned_pieces = {g: 0 for g in longs}
            shorts_done = [False]

            def try_scans(at_end):
                if not shorts_done[0] and NBLK - 1 in produced and NSH:
                    if NA:
                        scan_tier(tierA, SEGA, 0, NA * SEGA)
                    if NB:
                        scan_tier(tierB, SEGB, NA * SEGA, NB * SEGB)
                    shorts_done[0] = True
                for g in longs:
                    grps = pieces[g]
                    while scanned_pieces[g] < len(grps):
                        pi = scanned_pieces[g]
                        blks = grps[pi]
                        if not all(kb in produced for kb in blks):
                            break
                        scan_piece(g, pi, blks)
                        scanned_pieces[g] += 1

            for kb in order:
                produce(kb)
                produced.append(kb)
                try_scans(False)
            try_scans(True)
            assert shorts_done[0] or not NSH
            assert all(scanned_pieces[g] == len(pieces[g]) for g in longs)

            # final projection, all deferred here (PE is in-order); the
            # chunk with the tail scan goes last.
            fin_order = list(longs[1:]) + list(shorts) + [longs[0]]
            # bias enters as a rank-1 matmul: stationary = 1/128 columns,
            # moving = bias broadcast over partitions -> psy[b,o] = b_ho[o]
            nc.tensor.matmul(
                psy[:],
                bias_t[:, ds(O, BC)],
                bias_t[:, ds(0, O)],
                start=True,
                stop=False,
                skip_group_check=True,
            )
            for i, g in enumerate(fin_order):
                nc.scalar.copy(mh_t[:, ds(g * BC, BC)], m_t[:, ds(g * BC, BC)])
                nc.tensor.matmul(
                    psy[:],
                    mh_t[:, ds(g * BC, BC)],
                    who_t[:, ds(g * O, O)],
                    start=False,
                    stop=(i == NCH - 1),
                    skip_group_check=True,
                )

            y_t = consts.tile([BC, O], f32, tag="y", name="y_t")
            nc.scalar.copy(y_t[:], psy[:])
            nc.scalar.dma_start(Y, y_t[:])  # fast queue; sync's crawls
    nc.compile()
    return nc


def _get_program(plan):
    key = (plan["kgs"], plan["longs"])
    if key not in _CACHE:
        _CACHE[key] = _build(plan)
    return _CACHE[key]


def _ensure_ntff_hook():
    """Provide antenv.axon_hooks (absent in this image) so trace=True works."""
    import sys
    import types

    if "antenv.axon_hooks" in sys.modules:
        return True
    try:
        import antenv

        mod = types.ModuleType("antenv.axon_hooks")
        mod._hook = None

        def set_axon_ntff_profile_hook(h):
            mod._hook = h

        def get_axon_ntff_profile_hook():
            return mod._hook

        mod.set_axon_ntff_profile_hook = set_axon_ntff_profile_hook
        mod.get_axon_ntff_profile_hook = get_axon_ntff_profile_hook
        sys.modules["antenv.axon_hooks"] = mod
        antenv.axon_hooks = mod

        from trn_agent_boot.trn_boot import _ntff_profile_via_ctypes

        hook = _ntff_profile_via_ctypes("/opt/axon/libaxon_pjrt.so")
        mod.set_axon_ntff_profile_hook(hook)
        return hook is not None
    except Exception:
        return False


def kernel(X, W_ih, hh, W_ho, b_ho):
    from concourse import bass_utils

    X = np.asarray(X, dtype=np.float32)
    W_ih = np.asarray(W_ih, dtype=np.float32)
    hh = np.asarray(hh, dtype=np.float32)
    W_ho = np.asarray(W_ho, dtype=np.float32)
    b_ho = np.asarray(b_ho, dtype=np.float32)

    plan = _make_plan(hh)
    perm = plan["perm"]
    order = plan["order"]
    nc = _get_program(plan)

    wiht = np.ascontiguousarray(W_ih[perm].T).astype(np.float16)  # [I, H]
    wiht = np.ascontiguousarray(
        wiht.reshape(NI, 128, H).transpose(1, 0, 2).reshape(128, NI * H)
    )
    whot = np.ascontiguousarray(W_ho[:, perm].T).astype(np.float16)  # [H, O]
    whot = np.ascontiguousarray(
        whot.reshape(NCH, 128, O).transpose(1, 0, 2).reshape(128, NCH * O)
    )
    bias = np.zeros((128, O + BC), dtype=np.float16)
    bias[:, :O] = b_ho[None, :].astype(np.float16)
    bias[:, O:] = np.float16(1.0 / 128.0)

    common = {
        "WIHT": wiht,
        "WHOT": whot,
        "BIAS": bias,
        "SCN": plan["SCN"],
        "AUX": plan["AUX"],
    }
    in_maps = []
    for m in range(NCORES):
        im = dict(common)
        xm = X[:, m * BC : (m + 1) * BC, :]  # [S, BC, I]
        # per block: [NI, 128, BC*TB] -> row-major [128, NI*TB*BC]
        xt = xm.transpose(2, 1, 0).reshape(NI, 128, BC, NBLK, TB)
        xt = xt.transpose(3, 1, 0, 2, 4).reshape(NBLK, 128, XW)
        xt = xt[list(order)]  # production order
        im["X"] = np.ascontiguousarray(xt.transpose(1, 0, 2)).astype(np.float16)
        in_maps.append(im)

    trace = bool(int(os.environ.get("DIAG_TRACE", "0")))
    if trace:
        trace = _ensure_ntff_hook()
    res = None
    for attempt in range(3):
        try:
            res = bass_utils.run_bass_kernel_spmd(
                nc,
                in_maps,
                core_ids=list(range(NCORES)),
                trace=trace,
                tmpdir=os.environ.get("DIAG_TRACE_DIR") or None,
            )
            break
        except Exception:
            if attempt == 2:
                raise
            trace = False  # retry without profiling
    if res.exec_time_ns is not None:
        kernel.last_exec_time_ns = res.exec_time_ns
        kernel.last_mean_exec_time_ns = res.mean_exec_time_ns
    Yfull = np.concatenate([r["Y"] for r in res.results], axis=0)
    return Yfull


kernel.last_exec_time_ns = None
kernel.last_mean_exec_time_ns = None


# revision 55
# speedup vs baseline: 1.0296x; 1.0029x over previous
"""Trainium2 Bass kernel for nn_Diagnet (S=1024, B=64, I=512, H=2048, O=512).

    u = einsum('sbi,hi->sbh', X, W_ih)
    h_t = |u_t + hh * h_{t-1}|   (scan over S, only final h needed)
    Y = h_final @ W_ho.T + b_ho

Strategy (8 NeuronCores, data-parallel over batch, 8 batch rows/core):

* H lanes permuted so hh is sorted descending, split into 16 chunks of
  128.  Chunk g only needs the last kg steps where amax(g)^kg ~ 1e-5
  (truncation, exact far below fp16 noise).  kg rounds up to 64-step
  blocks; chunks with kg == 64 are "shorts" (window = final block only).
* GEMM in fp16 (X, W_ih host-cast).  PSUM fp32, then the Activation
  engine copies each [128, (b,tau)] tile into a per-column fp16 u
  buffer.  X arrives in 4 large row-contiguous DMAs (block 15's tile
  first so the end-of-sequence work unblocks early).
* The scan runs on the DVE as a custom instruction ABS_SCAN_ANT:
      state_k = |state_{k-1} - u_k * scn_k|   (ABSOLUTE_DIFF prefix scan)
  with scn = NEGATED pre-scales -a^(K-1-t), so state_k tracks the
  pre-scaled recurrence m_t = a^(K-1-t) h_t and the final element IS
  h_final.  A mask (Idx >= K-1) + ADD-accum extracts the final state
  into m[:, (g,b)], which is also the s0 carry for the next piece of
  the same column.  One instruction covers up to a whole window.
* Shorts all merge into ONE scan stream per batch column: segments of
  [24 flush elements (POSITIVE scales 128*2^-j fold any state to
  <1e-5) + 64-step window].  Uniform 88-element segments put every
  chunk's final state at stride 88; one strided DVE copy gathers them
  into m.
* Block production order: 15 FIRST (it ends every window -> shorts and
  mid chunks unblock early), then the mid chunks' remaining blocks,
  then 0..11 ascending for chunk 0's piece-chasing.  The tail after
  the last GEMM is chunk 0's last piece + the final projection.
* Final projection: per chunk, m -> fp16 on the Activation engine,
  matmul vs fp16 W_ho^T accumulated in one PSUM bank, all issued at
  the very end (the PE runs in order - issuing them early would stall
  the PE queue on the DVE scan pipeline); bias added on DVE.
"""

import math
import os

from contextlib import ExitStack

import numpy as np

S, B, I, H, O = 1024, 64, 512, 2048, 512
NCORES = 8
BC = B // NCORES  # 8 batch rows per core
TB = 64  # time block
NBLK = S // TB  # 16
NCH = H // 128  # 16 h-chunks
NI = I // 128  # 4 i-chunks
XW = NI * TB * BC  # 2048 X cols per block (ic-major)
LN_TRUNC = 11.5  # a^K <= e^-11.5 ~ 1e-5 -> truncate (gate is 2e-2)
NFLUSH = 16  # 32*2^-16 ~ 5e-4 residual after flush (<< fp16 noise)
FLUSH_M = 32.0  # fold start; shorts' h stays well under 32
WB = 16  # tier-B short window (chunks whose raw k <= WB)

_CACHE = {}


def _register_abs_scan():
    import concourse.dve_ops as dve_ops
    from concourse.dve_spec import Spec, Src0, Src1, Zero, C0, C1, scan, Idx, lower, AluOp
    from concourse.dve_uop import DveOpSpec

    for op in dve_ops.OPS:
        if op.name == "ABS_SCAN_ANT":
            return op

    def ref(in0, in1, s0, s1, imm2):
        x = in0.astype(np.float32) * in1.astype(np.float32)
        st = np.broadcast_to(np.asarray(s0, np.float32), x[:, 0].shape).copy()
        out = np.empty_like(x, dtype=np.float32)
        for k in range(x.shape[-1]):
            st = np.abs(st - x[:, k])
            out[:, k] = st * (k >= s1)
        return out

    state = scan(AluOp.ABSOLUTE_DIFF, Src0 * Src1, init=C0)
    spec = Spec(body=state * (Idx >= C1), accum=AluOp.ADD, accum_init=Zero, reference=ref)
    row = max(dve_ops._SUB_OPCODE_FOR_NAME.values()) + 1
    assert row < 0x20
    shas = {}
    for ver in ("v3", "v4"):
        s = DveOpSpec(name="ABS_SCAN_ANT", opcode=row, uops=lower(spec, ver=ver), rd1_en=True)
        shas[ver] = s.sha(ver)
    op = dve_ops.DveOp("ABS_SCAN_ANT", spec, subdim=False, uops_sha=shas)
    dve_ops._SUB_OPCODE_FOR_NAME["ABS_SCAN_ANT"] = row
    dve_ops.OPS.append(op)
    dve_ops.CUSTOM_DVE_SPECS["ABS_SCAN_ANT"] = spec
    return op


def _make_plan(hh):
    a = np.maximum(np.abs(hh.astype(np.float64)), 1e-30)
    perm = np.argsort(-a, kind="stable")
    a_s = a[perm]
    kgs = []
    for g in range(NCH):
        amax = a_s[g * 128]
        if amax >= math.exp(-LN_TRUNC / S):
            kg = S
        else:
            kg = min(S, int(math.ceil(LN_TRUNC / math.log(1.0 / amax))))
        kg = max(TB, min(S, ((kg + TB - 1) // TB) * TB))
        kgs.append(kg)
    assert all(kgs[g] >= kgs[g + 1] for g in range(NCH - 1)), kgs
    ag = a_s.reshape(NCH, 128)  # [chunk, lane]

    longs = [g for g in range(NCH) if kgs[g] > TB]
    shorts = [g for g in range(NCH) if kgs[g] == TB]
    NSH = len(shorts)

    # Two-tier shorts: tier A keeps the full 64-step window; tier B
    # (chunks whose raw truncation k <= WB) uses a WB-step window read
    # from the tail of block 15.  Each segment starts with NFLUSH flush
    # elements (POSITIVE scales FLUSH_M*2^-j fold any carry to <1e-3).
    kraw = {}
    for g in shorts:
        amax = a_s[g * 128]
        kraw[g] = min(S, int(math.ceil(LN_TRUNC / math.log(1.0 / max(amax, 1e-12)))))
    tierA = [g for g in shorts if kraw[g] > WB]
    tierB = [g for g in shorts if kraw[g] <= WB]
    NA, NB = len(tierA), len(tierB)
    SEGA = NFLUSH + TB
    SEGB = NFLUSH + WB
    sh_cols = NA * SEGA + NB * SEGB
    scn_off = {}
    off = 0
    for g in longs:
        scn_off[g] = off
        off += kgs[g]
    scn_shorts_off = off
    scn_cols = off + sh_cols
    sh = np.zeros((128, max(sh_cols, 1)), dtype=np.float64)
    flush = FLUSH_M * (0.5 ** np.arange(NFLUSH))
    for i, g in enumerate(tierA):
        base = i * SEGA
        sh[:, base : base + NFLUSH] = flush[None, :]
        t = np.arange(TB)
        sh[:, base + NFLUSH : base + SEGA] = -(ag[g][:, None] ** (TB - 1 - t)[None, :])
    for i, g in enumerate(tierB):
        base = NA * SEGA + i * SEGB
        sh[:, base : base + NFLUSH] = flush[None, :]
        t = np.arange(WB)
        sh[:, base + NFLUSH : base + SEGB] = -(ag[g][:, None] ** (WB - 1 - t)[None, :])
    scn = sh.astype(np.float32)
    # per-long-chunk generator constants: 1/a and -a^kg
    aux = np.zeros((128, 2 * len(longs)), dtype=np.float64)
    for j, g in enumerate(longs):
        aux[:, 2 * j] = 1.0 / ag[g]
        aux[:, 2 * j + 1] = -(ag[g] ** kgs[g])
    aux = aux.astype(np.float32)


    fb = {g: NBLK - kgs[g] // TB for g in longs}
    fb0 = fb[longs[0]]
    # block production order: 15 first (ends every window), then chunk 0's
    # early blocks INTERLEAVED with the mid chunks' blocks so chunk 0's
    # scan chain starts chasing production immediately (its scans are the
    # tail otherwise), then the rest of chunk 0 ascending.
    mids = longs[1:]
    mid_lo = min((fb[g] for g in mids), default=NBLK - 1)
    order = [NBLK - 1]
    order += [kb for kb in range(mid_lo, NBLK - 1)]
    order += [kb for kb in range(fb0, mid_lo)]
    assert sorted(order) == list(range(fb0, NBLK)), (order, fb)

    # scan pieces: mids = one piece (their blocks all produced early);
    # chunk 0 split so pieces chase production, last piece covers the
    # late-produced blocks in one go.
    pieces = {}
    for g in mids:
        pieces[g] = [list(range(fb[g], NBLK))]
    # mids scan only their exact truncation window (the 64-rounded u data
    # exists anyway; skipping the leading elements trims saturated DVE time)
    trim = {g: 0 for g in longs}
    for g in mids:
        amax = a_s[g * 128]
        kr = min(S, int(math.ceil(LN_TRUNC / math.log(1.0 / max(amax, 1e-12)))))
        trim[g] = max(0, kgs[g] - kr)
    # chunk 0: two pieces -- the first fires mid-stream and fills the
    # DVE idle gap before the last blocks are produced.
    nb0 = NBLK - fb0
    cut = fb0 + nb0 // 2
    if nb0 <= 6:
        pieces[longs[0]] = [list(range(fb0, NBLK))]
    else:
        pieces[longs[0]] = [list(range(fb0, cut)), list(range(cut, NBLK))]

    return {
        "perm": perm,
        "kgs": tuple(kgs),
        "longs": tuple(longs),
        "shorts": tuple(shorts),
        "scn_off": scn_off,
        "scn_shorts_off": scn_shorts_off,
        "tierA": tuple(tierA),
        "tierB": tuple(tierB),
        "sh_cols": sh_cols,
        "fb": fb,
        "order": tuple(order),
        "pieces": pieces,
        "trim": trim,
        "SCN": scn,
        "AUX": aux,
        "scn_cols": scn_cols,
    }


def _build(plan):
    import concourse.mybir as mybir
    import concourse.tile as tile
    from concourse import bacc
    from concourse.bass import ds

    ABS_SCAN = _register_abs_scan()
    f32 = mybir.dt.float32
    f16 = mybir.dt.float16

    kgs = plan["kgs"]
    longs = plan["longs"]
    shorts = plan["shorts"]
    tierA = plan["tierA"]
    tierB = plan["tierB"]
    NA, NB = len(tierA), len(tierB)
    SEGA = NFLUSH + TB
    SEGB = NFLUSH + WB
    sh_cols = plan["sh_cols"]
    NSH = len(shorts)
    scn_off = plan["scn_off"]
    scn_shorts_off = plan["scn_shorts_off"]
    fb = plan["fb"]
    order = plan["order"]
    pieces = plan["pieces"]
    trim = plan["trim"]
    scn_cols = plan["scn_cols"]
    NLG = len(longs)
    NORD = len(order)
    pos = {kb: i for i, kb in enumerate(order)}
    # map block -> (chunk g, piece index, offset-in-piece)
    piece_of = {}
    for g in longs:
        for pi, blks in enumerate(pieces[g]):
            for kb in blks:
                piece_of[(g, kb)] = (pi, blks.index(kb), len(blks))

    nc = bacc.Bacc("TRN2", target_bir_lowering=False, debug=False, num_devices=NCORES)
    # X rows are production-ordered: X[p, i, :] = block order[i], ic-major.
    X = nc.dram_tensor("X", [128, NORD, XW], f16, kind="ExternalInput").ap()
    WIHT = nc.dram_tensor("WIHT", [128, NI * H], f16, kind="ExternalInput").ap()
    WHOT = nc.dram_tensor("WHOT", [128, NCH * O], f16, kind="ExternalInput").ap()
    BIAS = nc.dram_tensor("BIAS", [128, O + BC], f16, kind="ExternalInput").ap()
    SCN = nc.dram_tensor("SCN", [128, plan["SCN"].shape[1]], f32, kind="ExternalInput").ap()
    AUX = nc.dram_tensor("AUX", [128, 2 * NLG], f32, kind="ExternalInput").ap()
    Y = nc.dram_tensor("Y", [BC, O], f32, kind="ExternalOutput").ap()

    with tile.TileContext(nc) as tc:
        with ExitStack() as ctx:
            consts = ctx.enter_context(tc.tile_pool(name="consts", bufs=1))
            gpool = ctx.enter_context(tc.tile_pool(name="gpsum", bufs=7, space="PSUM"))
            fpool = ctx.enter_context(tc.tile_pool(name="fpsum", bufs=1, space="PSUM"))

            # Dependency-granular tiles: one tile per DMA / per consumer so
            # Tile's coarse hazard tracking never falsely serializes.
            # Alternate the two HW DGE queues (SP is "sync", Act "scalar").
            qs = [nc.scalar, nc.sync]

            # Act queue gets ONLY a few early never-blocking DMAs (its
            # sequencer also runs the PSUM->SBUF copies; DMA-issue
            # instructions block on completion-semaphore reuse).  Sync
            # carries the bulk; its stalls block nothing.
            wih = [consts.tile([128, H], f16, tag=f"wih{ic}", name=f"wih{ic}") for ic in range(NI)]
            nc.scalar.dma_start(wih[0][:], WIHT[:, ds(0, H)])
            xt = {}
            for i, kb in enumerate(order):
                t = consts.tile([128, XW], f16, tag=f"xt{kb}", name=f"xt{kb}")
                xt[kb] = t
                (nc.scalar if i == 0 else nc.sync).dma_start(t[:], X[:, ds(i, 1), :])
                if i == 0:
                    if NSH:
                        scn_sh = consts.tile(
                            [128, sh_cols], f32, tag="scnsh", name="scn_sh"
                        )
                        nc.scalar.dma_start(scn_sh[:], SCN)
                    for ic in range(1, NI):
                        nc.sync.dma_start(wih[ic][:], WIHT[:, ds(ic * H, H)])
                    aux_t = consts.tile([128, 2 * NLG], f32, tag="aux", name="aux_t")
                    nc.sync.dma_start(aux_t[:], AUX)
            bias_t = consts.tile([128, O + BC], f16, tag="bias", name="bias_t")
            nc.sync.dma_start(bias_t[:], BIAS)
            who_t = consts.tile([128, NCH * O], f16, tag="whot", name="who_t")
            nc.sync.dma_start(who_t[:], WHOT)

            # u tiles: one per (long chunk, piece) + one for the shorts
            ut = {}
            for g in longs:
                for pi, blks in enumerate(pieces[g]):
                    plen = len(blks) * TB
                    ut[(g, pi)] = consts.tile(
                        [128, BC * plen], f16, tag=f"u{g}_{pi}", name=f"u{g}_{pi}"
                    )
            if NSH:
                ush = consts.tile([128, BC * sh_cols], f16, tag="ush", name="ush")
            m_t = consts.tile([128, NCH * BC], f32, tag="m", name="m_t")
            mh_t = consts.tile([128, NCH * BC], f16, tag="mh", name="mh_t")
            scr = consts.tile([128, max(sh_cols, S)], f32, tag="scr", name="scr")
            nc.vector.memset(m_t[:], 0.0)
            if NSH:
                # whole shorts u region -> 1.0: flush cells keep it (their
                # scales carry the flush constants); the copies overwrite
                # the window cells afterwards
                nc.gpsimd.memset(ush[:], 1.0)

            # long-chunk scales generated on-chip (geometric series)
            scn_lg = consts.tile([128, scn_cols - sh_cols], f32, tag="scnlg", name="scn_lg") if NLG else None
            zc = consts.tile([128, 1], f32, tag="zc", name="zc")
            nc.vector.memset(zc[:], 0.0)
            for j, g in enumerate(longs):
                kg = kgs[g]
                nc.vector.tensor_tensor_scan(
                    scn_lg[:, ds(scn_off[g], kg)],
                    aux_t[:, ds(2 * j, 1)].broadcast_to([128, kg]),
                    zc[:].broadcast_to([128, kg]),
                    aux_t[:, ds(2 * j + 1, 1)],
                    mybir.AluOpType.mult,
                    mybir.AluOpType.add,
                )

            psy = fpool.tile([BC, O], f32, tag="fy", name="psy")

            def produce(kb):
                active = [g for g in longs if fb[g] <= kb]
                if kb == NBLK - 1:
                    # shorts first: their copies unblock the merged scan
                    active = list(shorts) + active
                for lo in range(0, len(active), 3):
                    grp = active[lo : lo + 3]
                    ps = {
                        g: gpool.tile([128, TB * BC], f32, tag="gp", name=f"gp_{kb}_{g}")
                        for g in grp
                    }
                    for ic in range(NI):
                        for g in grp:
                            nc.tensor.matmul(
                                ps[g][:],
                                wih[ic][:, ds(g * 128, 128)],
                                xt[kb][:, ds(ic * TB * BC, TB * BC)],
                                start=(ic == 0),
                                stop=(ic == NI - 1),
                            )
                    for g in grp:
                        src = ps[g][:].rearrange("p (b t) -> p b t", b=BC)
                        if g in tierA:
                            i = tierA.index(g)
                            dst = ush[:].rearrange(
                                "p (b t) -> p b t", b=BC, t=sh_cols
                            )[:, :, ds(i * SEGA + NFLUSH, TB)]
                        elif g in tierB:
                            i = tierB.index(g)
                            dst = ush[:].rearrange(
                                "p (b t) -> p b t", b=BC, t=sh_cols
                            )[:, :, ds(NA * SEGA + i * SEGB + NFLUSH, WB)]
                            src = src[:, :, ds(TB - WB, WB)]
                        else:
                            pi, idx, nblk = piece_of[(g, kb)]
                            plen = nblk * TB
                            dst = ut[(g, pi)][:].rearrange(
                                "p (b t) -> p b t", b=BC, t=plen
                            )[:, :, ds(idx * TB, TB)]
                        nc.scalar.copy(dst, src)

            def scan_piece(g, pi, blks):
                lo = (blks[0] - fb[g]) * TB
                plen = len(blks) * TB  # u-tile column stride (full piece)
                skip = trim.get(g, 0) if pi == 0 else 0
                n = plen - skip
                for b in range(BC):
                    nc.vector._custom_dve(
                        ABS_SCAN,
                        out=scr[:, ds(0, n)],
                        in0=ut[(g, pi)][:, ds(b * plen + skip, n)],
                        in1=scn_lg[:, ds(scn_off[g] + lo + skip, n)],
                        s0=m_t[:, ds(g * BC + b, 1)],
                        s1=float(n - 1),
                        accum_out=m_t[:, ds(g * BC + b, 1)],
                    )

            def scan_tier(chunks, seg, col_lo, ncols):
                # one merged stream per batch column for a uniform-segment
                # tier; every segment starts with a flush, so the s0 carry
                # value is irrelevant and tiers are independent.
                cnt = len(chunks)
                g0 = chunks[0]
                for b in range(BC):
                    nc.vector._custom_dve(
                        ABS_SCAN,
                        out=scr[:, ds(0, ncols)],
                        in0=ush[:, ds(b * sh_cols + col_lo, ncols)],
                        in1=scn_sh[:, ds(col_lo, ncols)],
                        s0=m_t[:, ds(g0 * BC + b, 1)],
                        s1=float(-1.0),  # no mask; finals gathered from scr
                    )
                    src = scr[:, ds(0, ncols)].rearrange(
                        "p (s o) -> p s o", s=cnt, o=seg
                    )[:, :, ds(seg - 1, 1)]
                    dst = m_t[:, ds(g0 * BC, cnt * BC)].rearrange(
                        "p (s o) -> p s o", s=cnt, o=BC
                    )[:, :, ds(b, 1)]
                    nc.vector.tensor_scalar_mul(dst, src, 1.0)

            # ---- schedule ----
            produced = []
            scanned_pieces = {g: 0 for g in longs}
            shorts_done = [False]

            def try_scans(at_end):
                if not shorts_done[0] and NBLK - 1 in produced and NSH:
                    if NA:
                        h = (NA + 1) // 2
                        scan_tier(tierA[:h], SEGA, 0, h * SEGA)
                        if h < NA:
                            scan_tier(tierA[h:], SEGA, h * SEGA, (NA - h) * SEGA)
                    if NB:
                        scan_tier(tierB, SEGB, NA * SEGA, NB * SEGB)
                    shorts_done[0] = True
                for g in longs:
                    grps = pieces[g]
                    while scanned_pieces[g] < len(grps):
                        pi = scanned_pieces[g]
                        blks = grps[pi]
                        if not all(kb in produced for kb in blks):
                            break
                        scan_piece(g, pi, blks)
                        scanned_pieces[g] += 1

            for kb in order:
                produce(kb)
                produced.append(kb)
                try_scans(False)
            try_scans(True)
            assert shorts_done[0] or not NSH
            assert all(scanned_pieces[g] == len(pieces[g]) for g in longs)

            # final projection, all deferred here (PE is in-order); the
            # chunk with the tail scan goes last.
            fin_order = list(longs[1:]) + list(shorts) + [longs[0]]
            # bias enters as a rank-1 matmul: stationary = 1/128 columns,
            # moving = bias broadcast over partitions -> psy[b,o] = b_ho[o]
            nc.tensor.matmul(
                psy[:],
                bias_t[:, ds(O, BC)],
                bias_t[:, ds(0, O)],
                start=True,
                stop=False,
                skip_group_check=True,
            )
            for i, g in enumerate(fin_order):
                nc.scalar.copy(mh_t[:, ds(g * BC, BC)], m_t[:, ds(g * BC, BC)])
                nc.tensor.matmul(
                    psy[:],
                    mh_t[:, ds(g * BC, BC)],
                    who_t[:, ds(g * O, O)],
                    start=False,
                    stop=(i == NCH - 1),
                    skip_group_check=True,
                )

            y_t = consts.tile([BC, O], f32, tag="y", name="y_t")
            nc.scalar.copy(y_t[:], psy[:])
            nc.scalar.dma_start(Y, y_t[:])  # fast queue; sync's crawls
    nc.compile()
    return nc


def _get_program(plan):
    key = (plan["kgs"], plan["longs"])
    if key not in _CACHE:
        _CACHE[key] = _build(plan)
    return _CACHE[key]


def _ensure_ntff_hook():
    """Provide antenv.axon_hooks (absent in this image) so trace=True works."""
    import sys
    import types

    if "antenv.axon_hooks" in sys.modules:
        return True
    try:
        import antenv

        mod = types.ModuleType("antenv.axon_hooks")
        mod._hook = None

        def set_axon_ntff_profile_hook(h):
            mod._hook = h

        def get_axon_ntff_profile_hook():
            return mod._hook

        mod.set_axon_ntff_profile_hook = set_axon_ntff_profile_hook
        mod.get_axon_ntff_profile_hook = get_axon_ntff_profile_hook
        sys.modules["antenv.axon_hooks"] = mod
        antenv.axon_hooks = mod

        from trn_agent_boot.trn_boot import _ntff_profile_via_ctypes

        hook = _ntff_profile_via_ctypes("/opt/axon/libaxon_pjrt.so")
        mod.set_axon_ntff_profile_hook(hook)
        return hook is not None
    except Exception:
        return False


def kernel(X, W_ih, hh, W_ho, b_ho):
    from concourse import bass_utils

    X = np.asarray(X, dtype=np.float32)
    W_ih = np.asarray(W_ih, dtype=np.float32)
    hh = np.asarray(hh, dtype=np.float32)
    W_ho = np.asarray(W_ho, dtype=np.float32)
    b_ho = np.asarray(b_ho, dtype=np.float32)

    plan = _make_plan(hh)
    perm = plan["perm"]
    order = plan["order"]
    nc = _get_program(plan)

    wiht = np.ascontiguousarray(W_ih[perm].T).astype(np.float16)  # [I, H]
    wiht = np.ascontiguousarray(
        wiht.reshape(NI, 128, H).transpose(1, 0, 2).reshape(128, NI * H)
    )
    whot = np.ascontiguousarray(W_ho[:, perm].T).astype(np.float16)  # [H, O]
    whot = np.ascontiguousarray(
        whot.reshape(NCH, 128, O).transpose(1, 0, 2).reshape(128, NCH * O)
    )
    bias = np.zeros((128, O + BC), dtype=np.float16)
    bias[:, :O] = b_ho[None, :].astype(np.float16)
    bias[:, O:] = np.float16(1.0 / 128.0)

    common = {
        "WIHT": wiht,
        "WHOT": whot,
        "BIAS": bias,
        "SCN": plan["SCN"],
        "AUX": plan["AUX"],
    }
    in_maps = []
    for m in range(NCORES):
        im = dict(common)
        xm = X[:, m * BC : (m + 1) * BC, :]  # [S, BC, I]
        # per block: [NI, 128, BC*TB] -> row-major [128, NI*TB*BC]
        xt = xm.transpose(2, 1, 0).reshape(NI, 128, BC, NBLK, TB)
        xt = xt.transpose(3, 1, 0, 2, 4).reshape(NBLK, 128, XW)
        xt = xt[list(order)]  # production order
        im["X"] = np.ascontiguousarray(xt.transpose(1, 0, 2)).astype(np.float16)
        in_maps.append(im)

    trace = bool(int(os.environ.get("DIAG_TRACE", "0")))
    if trace:
        trace = _ensure_ntff_hook()
    res = None
    for attempt in range(3):
        try:
            res = bass_utils.run_bass_kernel_spmd(
                nc,
                in_maps,
                core_ids=list(range(NCORES)),
                trace=trace,
                tmpdir=os.environ.get("DIAG_TRACE_DIR") or None,
            )
            break
        except Exception:
            if attempt == 2:
                raise
            trace = False  # retry without profiling
    if res.exec_time_ns is not None:
        kernel.last_exec_time_ns = res.exec_time_ns
        kernel.last_mean_exec_time_ns = res.mean_exec_time_ns
    Yfull = np.concatenate([r["Y"] for r in res.results], axis=0)
    return Yfull


kernel.last_exec_time_ns = None
kernel.last_mean_exec_time_ns = None
